# revision 17
# baseline (speedup 1.0000x reference)
"""KMeans assignment kernel for Trainium2 (8 NeuronCores, SPMD data-parallel).

Problem: x [8, 4096, 1024] f32, C [1024, 4096] f32, Cnorm [1, 4096] f32.
Output: argmin_k(|x|^2 - 2 x.C + Cnorm) as int32 [8, 4096].

Strategy:
  - |x|^2 is row-constant, so argmin(dist) == argmax(x.C - 0.5*Cnorm).
  - Shard rows (N = B*T = 32768) across 8 cores, 4096 rows each; replicate C.
  - Single-pass bf16 matmul (f32 PSUM accumulate).  bf16 input quantization
    gives score noise sigma ~ 0.075; rows whose top1-top2 margin < TAU are
    recomputed exactly on the host with the reference's jax-on-CPU numerics
    (~5% of rows, one small sgemm).  bf16 also halves DMA traffic and drops
    the LDWEIGHTS time under the 213 ns matmul streaming floor (f32r weights
    load at 187 ns and gate the issue cadence at 227 ns).
  - C (16 chunks' worth, 8 MB bf16) is fully SBUF-resident but streamed
    n-major; a warm block revisits 4 cached x tiles against each arriving
    2 MB chunk (chunk 0 t-outer chasing slice arrivals) so the PE saturates
    ~8 us in and the HAM cold ramp hides under DMA pacing.  Afterwards each
    remaining x tile gets one full-width 8-bank visit, x read exactly once.
  - ALL DMA triggers ride the sync queue: a trigger costs ~0.6 us on the
    issuing engine and is flow-controlled, so triggers must never be issued
    from an engine with drain work (that serialization cost 39 us/run).
  - Each PSUM bank is drained right after its 8-matmul accumulation by a
    3-engine chain sized to hide under the 8-MM fill: ACT copies PSUM->SBUF
    (702 ns), GPSIMD subtracts 0.5*Cnorm (1293 ns), DVE does top-8 MAX8 +
    FIND_INDEX8 on 512 elems (~1.4 us).  Per (tile, bank) the top-8 values +
    local indices land in a candidate buffer; the host merges the 64
    candidates per row (same argmax, exact margins).  This replaces the
    full-4096 DVE argmax per tile and the 40 us serial tail of the original.
"""

import os
import sys

import numpy as np
import ml_dtypes

for _p in ("/opt/trn_rl_repo",):
    if os.path.isdir(_p) and _p not in sys.path:
        sys.path.insert(0, _p)

import concourse.bass as bass
import concourse.mybir as mybir
import concourse.tile as tile
from concourse import bacc
from concourse.bass_utils import run_bass_kernel_spmd

B, T, D, K = 8, 4096, 1024, 4096
N_CORES = 8
ROWS = (B * T) // N_CORES  # 4096 rows per core
P = 128  # SBUF partitions / PE tile
MT = ROWS // P  # 32 row-tiles per core
DC = D // P  # 8 contraction chunks
NB = 512  # matmul free dim = one PSUM bank of f32
NC_ = K // NB  # 8 centroid chunks
WARM = 4  # x tiles revisited per C chunk during the DMA-paced warm block

TAU = 0.85  # margin flag threshold: bf16 noise (sigma~0.075) + encode quantization
MAGIC2 = 786432.0  # 1.5 * 2^19: fixed-point-aligns f32 scores to a 2^-4 grid
STT_A = 1536.0 - MAGIC2  # fused re-bias: enc = (t1 + STT_A) - T2[k]

_compiled = {}


def _build():
    nc = bacc.Bacc("TRN2", target_bir_lowering=False, debug=False, num_devices=N_CORES)

    x_d = nc.dram_tensor("x", [MT, DC, P, P], mybir.dt.bfloat16, kind="ExternalInput")
    c_d = nc.dram_tensor("c", [DC, P, K], mybir.dt.bfloat16, kind="ExternalInput")
    cn_d = nc.dram_tensor("cn", [P, K], mybir.dt.float32, kind="ExternalInput")
    tg_d = nc.dram_tensor("tg", [P, K], mybir.dt.float32, kind="ExternalInput")
    io_d = nc.dram_tensor("io", [P, NB], mybir.dt.float32, kind="ExternalInput")
    cv_d = nc.dram_tensor("cv", [MT, P, NC_ * 8], mybir.dt.float32, kind="ExternalOutput")

    with tile.TileContext(nc) as tc:
        with (
            tc.tile_pool(name="const", bufs=1) as cpool,
            tc.tile_pool(name="xp", bufs=WARM + 3) as xpool,
            tc.tile_pool(name="t1p", bufs=6) as apool,
            tc.tile_pool(name="s2p", bufs=6) as gpool,
            tc.tile_pool(name="ps", bufs=8, space=bass.MemorySpace.PSUM) as ppool,
        ):
            c_sb = cpool.tile([P, DC, K], mybir.dt.bfloat16, tag="c")
            cn_sb = cpool.tile([P, K], mybir.dt.float32, tag="cn")
            tg_sb = cpool.tile([P, K], mybir.dt.float32, tag="tg")
            io_sb = cpool.tile([P, NB], mybir.dt.float32, tag="io")
            cv_sb = cpool.tile([P, MT * NC_ * 8], mybir.dt.float32, tag="cv")

            def drain(t, n, ps, use_gpsimd=False):
                """Drain one PSUM bank: ACT copies PSUM->SBUF with a +MAGIC2
                bias (aligns scores to the 2^-4 grid), then either DVE's fused
                scalar_tensor_tensor re-biases and subtracts
                T2[k] = round_q(0.5|C_k|^2) - 512 - j_local*2^-13, or (for a
                subset of banks, to balance engine load) GPSIMD subtracts the
                index-free table tg[k] = MAGIC2 - 1536 + round_q(..) - 512 and
                adds the iota tile -- bit-identical results.  Each score's 9
                low mantissa bits end up stamped with its bank-local index, so
                DVE only needs MAX8: the top-8 candidate VALUES carry their
                indices and the host decodes the bits."""
                sl = slice(n * NB, (n + 1) * NB)
                t1 = apool.tile([P, NB], mybir.dt.float32, tag="t1")
                nc.scalar.activation(
                    t1[:], ps[:], mybir.ActivationFunctionType.Copy, bias=MAGIC2
                )
                s2 = gpool.tile([P, NB], mybir.dt.float32, tag="s2")
                if use_gpsimd:
                    u2 = gpool.tile([P, NB], mybir.dt.float32, tag="u2")
                    nc.gpsimd.tensor_sub(u2[:], t1[:], tg_sb[:, sl])
                    nc.gpsimd.tensor_add(s2[:], u2[:], io_sb[:])
                else:
                    nc.vector.scalar_tensor_tensor(
                        s2[:], t1[:], STT_A, cn_sb[:, sl],
                        mybir.AluOpType.add, mybir.AluOpType.subtract,
                    )
                co = t * (NC_ * 8) + n * 8
                nc.vector.max(out=cv_sb[:, co : co + 8], in_=s2[:])

            def cand_out(t):
                co = t * (NC_ * 8)
                nc.sync.dma_start(out=cv_d[t], in_=cv_sb[:, co : co + NC_ * 8])

            # x tiles + candidate outputs ride the sync DMA queue; C + Cnorm
            # stream n-major on the scalar engine's queue in parallel.
            # DMA triggers cost ~0.6 us on the issuing engine and are
            # flow-controlled, so they must never sit in front of drain work:
            # everything rides the sync queue (the sync engine does nothing
            # else), interleaved [C chunk n, cn slice n] so each 2.25 MB
            # arrives in ~6.5 us -- just ahead of the 7.3 us the PE needs it.
            xw = [
                xpool.tile([P, DC, P], mybir.dt.bfloat16, tag="x", name=f"xw{t}")
                for t in range(WARM)
            ]
            nc.sync.dma_start(out=xw[0][:], in_=x_d[0].rearrange("c p j -> p c j"))
            sl0 = slice(0, NB)
            for h in range(2):  # chunk 0 in two 4-slice triggers (prologue)
                cs = slice(h * DC // 2, (h + 1) * DC // 2)
                nc.sync.dma_start(
                    out=c_sb[:, cs, sl0],
                    in_=c_d[cs, :, sl0].rearrange("c p j -> p c j"),
                )
            for t in range(1, WARM):
                nc.sync.dma_start(out=xw[t][:], in_=x_d[t].rearrange("c p j -> p c j"))
            nc.sync.dma_start(out=cn_sb[:, sl0], in_=cn_d[:, sl0])
            nc.sync.dma_start(out=tg_sb[:, sl0], in_=tg_d[:, sl0])
            nc.sync.dma_start(out=io_sb[:], in_=io_d[:])
            for n in range(1, NC_):
                sl = slice(n * NB, (n + 1) * NB)
                nc.sync.dma_start(
                    out=c_sb[:, :, sl], in_=c_d[:, :, sl].rearrange("c p j -> p c j")
                )
                nc.sync.dma_start(out=cn_sb[:, sl], in_=cn_d[:, sl])
                nc.sync.dma_start(out=tg_sb[:, sl], in_=tg_d[:, sl])

            # Warm block: revisit the WARM cached x tiles against each C chunk
            # as it arrives (chunk 0 goes t-outer chasing slice arrivals; later
            # chunks go c-outer / t-inner; the HAM cold ramp and x prefetch
            # hide under DMA pacing).
            for n in range(NC_):
                sl = slice(n * NB, (n + 1) * NB)
                ps = [
                    ppool.tile([P, NB], mybir.dt.float32, tag="ps", name=f"psw{n}_{ti}")
                    for ti in range(WARM)
                ]
                order = (
                    [(c, ti) for ti in range(WARM) for c in range(DC)]
                    if n == 0
                    else [(c, ti) for c in range(DC) for ti in range(WARM)]
                )
                for c, ti in order:
                    nc.tensor.matmul(
                        ps[ti][:],
                        xw[ti][:, c, :],
                        c_sb[:, c, sl],
                        start=(c == 0),
                        stop=(c == DC - 1),
                    )
                for ti in range(WARM):
                    drain(ti, n, ps[ti], use_gpsimd=(ti < 2))
                    # cand_out for warm tiles is deferred into the first main
                    # visits so its DVE-wait never blocks the sync queue here.

            # Main loop: one full-width visit (all 8 banks) per remaining tile.
            for t in range(WARM, MT):
                x_sb = xpool.tile([P, DC, P], mybir.dt.bfloat16, tag="x")
                nc.sync.dma_start(out=x_sb[:], in_=x_d[t].rearrange("c p j -> p c j"))
                if t - WARM < WARM:
                    cand_out(t - WARM)  # deferred warm-tile output
                ps = [
                    ppool.tile([P, NB], mybir.dt.float32, tag="ps", name=f"ps{t}_{n}")
                    for n in range(NC_)
                ]
                for n in range(NC_):
                    sl = slice(n * NB, (n + 1) * NB)
                    for c in range(DC):
                        nc.tensor.matmul(
                            ps[n][:],
                            x_sb[:, c, :],
                            c_sb[:, c, sl],
                            start=(c == 0),
                            stop=(c == DC - 1),
                        )
                    drain(t, n, ps[n], use_gpsimd=(n < 3))
                cand_out(t)

    nc.compile()
    return nc


def _xt_tiles(xs):
    # [r, d] -> [m, c, p, j] with r = m*128 + j, d = c*128 + p
    return np.ascontiguousarray(
        xs.astype(ml_dtypes.bfloat16).reshape(MT, P, DC, P).transpose(0, 2, 3, 1)
    )


def _host_fixup(assigned, margins, x2, Cf, Cnorm):
    """Recompute rows whose fp22 score margin is within noise of a tie,
    replicating the reference's jax-on-CPU f32 numerics exactly."""
    bad = np.flatnonzero(margins < TAU)
    if bad.size == 0:
        return assigned
    import jax
    import jax.numpy as jnp

    cpu = jax.devices("cpu")[0]
    with jax.default_device(cpu):
        xb = jnp.asarray(x2[bad])
        Cj = jnp.asarray(Cf)
        cnj = jnp.asarray(Cnorm.reshape(1, K))
        dist = jnp.sum(xb * xb, axis=1, keepdims=True) - 2.0 * (xb @ Cj) + cnj
        fixed = np.asarray(jnp.argmin(dist, axis=1), dtype=assigned.dtype)
    assigned[bad] = fixed
    return assigned


def run(inputs, trace=False, mode=None):
    """Returns (assigned [B, T] int32, BassKernelResults)."""
    if "k" not in _compiled:
        _compiled["k"] = _build()
    nc = _compiled["k"]

    x2 = np.ascontiguousarray(
        np.asarray(inputs["x"], dtype=np.float32).reshape(B * T, D)
    )
    Cf = np.ascontiguousarray(np.asarray(inputs["C"], dtype=np.float32))
    Cnorm = np.asarray(inputs["Cnorm"], dtype=np.float32)
    cn_q = np.round(0.5 * Cnorm.reshape(K) * 16.0).astype(np.float32) / np.float32(16.0)
    jloc = (np.arange(K) % NB).astype(np.float32)
    t2 = (cn_q - np.float32(512.0) - jloc * np.float32(2.0**-13)).astype(np.float32)
    cn = np.ascontiguousarray(np.broadcast_to(t2.reshape(1, K), (P, K)))
    tgt = (np.float32(MAGIC2 - 1536.0) + cn_q - np.float32(512.0)).astype(np.float32)
    tg = np.ascontiguousarray(np.broadcast_to(tgt.reshape(1, K), (P, K)))
    io = np.ascontiguousarray(np.broadcast_to(
        (np.arange(NB).astype(np.float32) * np.float32(2.0**-13)).reshape(1, NB), (P, NB)
    ))
    c3 = np.ascontiguousarray(Cf.astype(ml_dtypes.bfloat16).reshape(DC, P, K))

    in_maps = []
    for s in range(N_CORES):
        xs = x2[s * ROWS : (s + 1) * ROWS]
        in_maps.append({"x": _xt_tiles(xs), "c": c3, "cn": cn, "tg": tg, "io": io})

    res = run_bass_kernel_spmd(nc, in_maps, list(range(N_CORES)), trace=trace)

    parts = []
    margins = []
    for s in range(N_CORES):
        cv = np.ascontiguousarray(np.asarray(res.results[s]["cv"]).reshape(ROWS, NC_ * 8))
        slot = np.argmax(cv, axis=1)
        r = np.arange(ROWS)
        j = cv.view(np.uint32)[r, slot] & 0x1FF
        idx = (slot >> 3) * NB + j
        top2 = np.partition(cv, NC_ * 8 - 2, axis=1)[:, -2:]
        margins.append(top2[:, 1] - top2[:, 0])
        parts.append(idx.astype(np.int32))

    assigned = np.concatenate(parts)
    margins = np.concatenate(margins)
    assigned = _host_fixup(assigned, margins, x2, Cf, Cnorm)
    return assigned.reshape(B, T), res


def kernel(x, C, Cnorm):
    assigned, _ = run({"x": x, "C": C, "Cnorm": Cnorm})
    return assigned
